# revision 17
# baseline (speedup 1.0000x reference)
"""Trainium2 Bass kernel for 3-layer residual LSTM decoder (B=64,T=1024,H=768).

Sharding: layer-pipeline across cores 0/1/2 (one LSTM layer each); all 8 cores
share the bulk input-projection GEMMs (each computes a 384-col slice of every
layer's pre-activations). Two AllGathers per 32-step window move x-chunks
downstream and distribute the pre slices. SPMD program: per-core behavior
differs only through input data and partition_id-derived indices.

Per step (batch=64 on partitions): 6 fp32r matmuls accumulate h@WhhT into
PSUM per 1024-col gate block (gate columns permuted so each block holds
i/f/g/o for one 256-wide h slice), a bf16 identity matmul adds the
precomputed input term, ACT applies sigmoid/tanh from PSUM, DVE updates c/h,
PE re-transposes h into the stationary layout for the next step.
"""

import numpy as np
import ml_dtypes

import concourse.bass as bass
import concourse.tile as tile
from concourse import bacc, mybir
from concourse import bass_utils

F32 = mybir.dt.float32
F32R = mybir.dt.float32r
BF16 = mybir.dt.bfloat16

B = 64
T_FULL = 1024
IN = 512
H = 768
G = 4 * H          # 3072
OUT = 100
NCORES = 8

HC = H // 128      # 6
KIN = IN // 128    # 4
NBLK = 3
BLKW = G // NBLK   # 1024
SLICE = G // NCORES  # 384
SKEW = 5

Sig = mybir.ActivationFunctionType.Sigmoid
Tanh = mybir.ActivationFunctionType.Tanh


def gate_perm():
    """perm[n] = original gate column of permuted column n. Block bb holds
    [i,f,g,o] each 256 wide for h-cols [256bb, 256bb+256)."""
    n = np.arange(G)
    bb = n // BLKW
    r = n % BLKW
    q = r // 256
    m = r % 256
    return (q * H + bb * 256 + m).astype(np.int64)


def build_kernel(ws=32, t=T_FULL, unroll=8, skip_ag=False):
    nwin = t // ws
    nprog = nwin + SKEW
    rows_w = B * ws            # rows per window
    mt = rows_w // 128         # M tiles per window

    nc = bacc.Bacc("TRN2", target_bir_lowering=False, debug=False,
                   num_devices=NCORES)

    xT = nc.dram_tensor("xT", [IN, B * t], BF16, kind="ExternalInput")
    whhT = nc.dram_tensor("whhT", [H, G], F32R, kind="ExternalInput")
    wih1 = nc.dram_tensor("wih1", [IN, SLICE], BF16, kind="ExternalInput")
    wih2 = nc.dram_tensor("wih2", [H, SLICE], BF16, kind="ExternalInput")
    wih3 = nc.dram_tensor("wih3", [H, SLICE], BF16, kind="ExternalInput")
    biases = nc.dram_tensor("biases", [1, 3, SLICE], BF16, kind="ExternalInput")
    wpT = nc.dram_tensor("wpT", [H, OUT], BF16, kind="ExternalInput")
    ident = nc.dram_tensor("ident", [B, B], BF16, kind="ExternalInput")
    identf = nc.dram_tensor("identf", [B, B], F32, kind="ExternalInput")
    alpha = nc.dram_tensor("alpha", [128, 1], F32, kind="ExternalInput")
    hscale = nc.dram_tensor("hscale", [128, nprog], F32, kind="ExternalInput")
    out = nc.dram_tensor("out", [B * t, OUT], F32, kind="ExternalOutput")
    scratch_out = nc.dram_tensor("scratch_out", [rows_w, OUT], F32,
                                 kind="Internal")

    with tile.TileContext(nc) as tc:
        with (
            tc.tile_pool(name="const", bufs=1) as constp,
            tc.tile_pool(name="state", bufs=1) as statep,
            tc.tile_pool(name="pre", bufs=2) as prep,
            tc.tile_pool(name="gact", bufs=2) as gactp,
            tc.tile_pool(name="small", bufs=2) as smallp,
            tc.tile_pool(name="lhst", bufs=4) as lhstp,
            tc.tile_pool(name="bulko", bufs=4) as bulkop,
            tc.tile_pool(name="gpsum", bufs=2, space="PSUM") as gpsump,
            tc.tile_pool(name="tpsum", bufs=2, space="PSUM") as tpsump,
            tc.tile_pool(name="bpsum", bufs=2, space="PSUM") as bpsump,
            tc.tile_pool(name="dram", bufs=2, space="DRAM") as dramp,
        ):
            # ---------------- persistent SBUF ----------------
            whh_sb = constp.tile([128, HC, G], F32R)
            for kc in range(HC):
                nc.sync.dma_start(whh_sb[:, kc, :],
                                  whhT[128 * kc:128 * (kc + 1), :])
            wih1_sb = constp.tile([128, KIN, SLICE], BF16)
            for kc in range(KIN):
                nc.sync.dma_start(wih1_sb[:, kc, :],
                                  wih1[128 * kc:128 * (kc + 1), :])
            wih2_sb = constp.tile([128, HC, SLICE], BF16)
            wih3_sb = constp.tile([128, HC, SLICE], BF16)
            for kc in range(HC):
                nc.sync.dma_start(wih2_sb[:, kc, :],
                                  wih2[128 * kc:128 * (kc + 1), :])
                nc.sync.dma_start(wih3_sb[:, kc, :],
                                  wih3[128 * kc:128 * (kc + 1), :])
            bias_sb = constp.tile([1, 3, SLICE], BF16)
            nc.sync.dma_start(bias_sb[:], biases[:])
            wp_sb = constp.tile([128, HC, OUT], BF16)
            for kc in range(HC):
                nc.sync.dma_start(wp_sb[:, kc, :],
                                  wpT[128 * kc:128 * (kc + 1), :])
            id_sb = constp.tile([B, B], BF16)
            nc.sync.dma_start(id_sb[:], ident[:])
            idf_sb = constp.tile([B, B], F32)
            nc.sync.dma_start(idf_sb[:], identf[:])
            ones_sb = constp.tile([1, 128], BF16)
            nc.vector.memset(ones_sb[:], 1.0)
            alpha_sb = constp.tile([128, 1], F32)
            nc.sync.dma_start(alpha_sb[:], alpha[:])
            hscale_sb = constp.tile([128, nprog], F32)
            nc.sync.dma_start(hscale_sb[:], hscale[:])

            c_sb = statep.tile([B, NBLK, 256], F32)
            nc.vector.memset(c_sb[:], 0.0)
            hT_sb = statep.tile([128, HC, B], F32)
            nc.vector.memset(hT_sb[:], 0.0)
            hTr_st = statep.tile([128, HC, B], F32R)
            nc.vector.tensor_copy(hTr_st[:], hT_sb[:])
            xout_win = statep.tile([128, HC, rows_w], BF16)

            sp_eng = bass.OrderedSet([mybir.EngineType.SP])
            pid = nc.partition_id(engines=sp_eng)
            lid = nc.snap((pid % 3), engines=sp_eng, min_val=0, max_val=2)
            res_sh = nc.snap((pid + 7) % 8, engines=sp_eng, min_val=0,
                             max_val=7)

            # ------------- AG ring: parity 2 -------------
            agp_in = [dramp.tile([3, rows_w, SLICE], BF16, tag="agp_in",
                                 name=f"agp_in{i}") for i in range(2)]
            agp_out = [dramp.tile([NCORES, 3, rows_w, SLICE], BF16,
                                  tag="agp_out", name=f"agp_out{i}") for i in range(2)]
            agx_in = [dramp.tile([HC, 128, rows_w], BF16, tag="agx_in",
                                 name=f"agx_in{i}") for i in range(2)]
            agx_out = [dramp.tile([NCORES, HC, 128, rows_w], BF16,
                                  tag="agx_out", name=f"agx_out{i}") for i in range(2)]

            # one-time zeroing of the AG ring (junk must be finite: 0*NaN=NaN
            # would defeat the hscale state reset)
            ztile = constp.tile([128, 2048], BF16)
            nc.vector.memset(ztile[:], 0.0)
            for buf in agp_in + agx_in:
                v = buf[:].flatten().rearrange("(p n) -> p n", p=128)
                ncols = v.shape[1]
                off = 0
                while off < ncols:
                    w = min(2048, ncols - off)
                    nc.sync.dma_start(v[:, off:off + w], ztile[:, 0:w])
                    off += w

            def bulk_gemm(lgi, n_kc, lhs_loader, wsb, nagpi):
                """my pre-slice of layer lgi for one window -> next AG input."""
                for m in range(mt):
                    ps = bpsump.tile([128, SLICE], F32, tag="bps")
                    lhm = lhs_loader(m)
                    for kc in range(n_kc):
                        nc.tensor.matmul(ps[:], lhm[:, kc, :], wsb[:, kc, :],
                                         start=(kc == 0), stop=False)
                    nc.tensor.matmul(ps[:], ones_sb[:], bias_sb[:, lgi, :],
                                     start=False, stop=True)
                    ob = bulkop.tile([128, SLICE], BF16, tag="bob")
                    nc.vector.tensor_copy(ob[:], ps[:])
                    nc.sync.dma_start(nagpi[lgi, 128 * m:128 * (m + 1), :],
                                      ob[:])

            def xt_loader(p):
                col0 = min(p, nwin - 1) * rows_w
                xv = xT[:].rearrange("(a p) c -> a p c", p=128)
                def load_m(m):
                    lh = lhstp.tile([128, HC, 128], BF16, tag="lh")
                    nc.sync.dma_start(
                        lh[:, 0:KIN, :],
                        xv[:, :, col0 + 128 * m:col0 + 128 * (m + 1)]
                        .transpose([1, 0, 2]))
                    return lh
                return load_m

            def agx_tile_loader(ago, slot):
                def load_m(m):
                    lh = lhstp.tile([128, HC, 128], BF16, tag="lh")
                    nc.sync.dma_start(
                        lh[:],
                        ago[slot, 0:HC, :, 128 * m:128 * (m + 1)]
                        .transpose([1, 0, 2]))
                    return lh
                return load_m

            # ---------------- program windows ----------------
            for p in range(nprog):
                par = p % 2
                ago_p = agp_out[par]
                ago_x = agx_out[par]
                ago_x_prev = agx_out[1 - par]

                if p >= 1 and not skip_ag:
                    nc.gpsimd.collective_compute(
                        "AllGather", mybir.AluOpType.bypass,
                        replica_groups=[list(range(NCORES))],
                        ins=[agp_in[1 - par].opt()], outs=[ago_p.opt()])
                    nc.gpsimd.collective_compute(
                        "AllGather", mybir.AluOpType.bypass,
                        replica_groups=[list(range(NCORES))],
                        ins=[agx_in[1 - par].opt()], outs=[ago_x.opt()])

                # scale state at window start (zero at my first real window)
                nc.vector.tensor_scalar_mul(hT_sb[:], hT_sb[:],
                                            hscale_sb[:, p:p + 1])
                nc.vector.tensor_copy(hTr_st[:], hT_sb[:])
                nc.vector.tensor_scalar_mul(c_sb[:], c_sb[:],
                                            hscale_sb[0:B, p:p + 1])

                def emit_step(tv):
                    # pre(t): 8 slices (permuted-contiguous) at my layer idx
                    pre_sb = prep.tile([B, NCORES, SLICE], BF16, tag="pre")
                    if p == 0:
                        # agp_out not AG-written yet; any finite value works
                        nc.vector.memset(pre_sb[:], 0.0)
                    else:
                        nc.sync.dma_start(
                            pre_sb[:],
                            ago_p[0:NCORES, bass.ds(lid, 1), bass.ts(tv, B), :]
                            .transpose([2, 0, 1, 3]).squeeze(2))

                    gact = gactp.tile([B, NBLK, BLKW], F32, tag="gact")
                    hfull = gactp.tile([B, NBLK, 256], F32, tag="hfull")
                    for bb in range(NBLK):
                        ps = gpsump.tile([B, BLKW], F32, tag="gps")
                        for k in range(HC):
                            lh = hTr_st[:, k, :]
                            for nh in range(2):
                                nc.tensor.matmul(
                                    ps[:, 512 * nh:512 * (nh + 1)], lh,
                                    whh_sb[:, k, BLKW * bb + 512 * nh:
                                           BLKW * bb + 512 * (nh + 1)],
                                    start=(k == 0), stop=False)
                        for nh in range(2):
                            nc.tensor.matmul(
                                ps[:, 512 * nh:512 * (nh + 1)], id_sb[:],
                                pre_sb[:, :, :].rearrange(
                                    "b s l -> b (s l)")[:, BLKW * bb + 512 * nh:
                                                        BLKW * bb + 512 * (nh + 1)],
                                start=False, stop=True)
                        gb = gact[:, bb, :]
                        nc.scalar.activation(gb[:, 0:512], ps[:, 0:512], Sig)
                        nc.scalar.activation(gb[:, 512:768], ps[:, 512:768],
                                             Tanh)
                        nc.scalar.activation(gb[:, 768:1024], ps[:, 768:1024],
                                             Sig)
                        # c/h tail for this block (overlaps next block's MMs)
                        igb = smallp.tile([B, 256], F32, tag="ig")
                        nc.vector.tensor_mul(igb[:], gb[:, 0:256],
                                             gb[:, 512:768])
                        nc.vector.tensor_mul(c_sb[:, bb, :], gb[:, 256:512],
                                             c_sb[:, bb, :])
                        nc.vector.tensor_add(c_sb[:, bb, :], c_sb[:, bb, :],
                                             igb[:])
                        tcb = smallp.tile([B, 256], F32, tag="tct")
                        nc.scalar.activation(tcb[:], c_sb[:, bb, :], Tanh)
                        nc.vector.tensor_mul(hfull[:, bb, :], gb[:, 768:1024],
                                             tcb[:])

                    # transposes last so they don't split the PE MM stream
                    for bb in range(NBLK):
                        tp = tpsump.tile([128, 2, B], F32, tag="tp")
                        for j in range(2):
                            nc.tensor.transpose(
                                tp[:, j, :],
                                hfull[:, bb, 128 * j:128 * (j + 1)],
                                idf_sb[:])
                        nc.vector.tensor_copy(hT_sb[:, 2 * bb:2 * bb + 2, :],
                                              tp[:])
                        nc.vector.tensor_copy(hTr_st[:, 2 * bb:2 * bb + 2, :],
                                              tp[:])

                    # xout(t) = alpha*res(t) + hT on GpSimd (off DVE FIFO)
                    res_sb = prep.tile([128, HC, B], BF16, tag="res")
                    if p < 2:
                        nc.vector.memset(res_sb[:], 0.0)
                    else:
                        nc.sync.dma_start(
                            res_sb[:],
                            ago_x_prev[bass.ds(res_sh, 1), 0:HC, :,
                                       bass.ts(tv, B)]
                            .transpose([2, 1, 0, 3]).squeeze(2))
                    nc.vector.scalar_tensor_tensor(
                        xout_win[:, :, bass.ts(tv, B)],
                        res_sb[:],
                        alpha_sb[:, 0:1],
                        hT_sb[:],
                        mybir.AluOpType.mult, mybir.AluOpType.add)

                if unroll >= ws:
                    for tv in range(ws):
                        emit_step(tv)
                else:
                    tc.For_i_unrolled(0, ws, 1, emit_step, unroll)

                # contribute xout window to next AG input
                nagx = agx_in[par]
                nagp = agp_in[par]
                for kc in range(HC):
                    nc.sync.dma_start(nagx[kc, :, :], xout_win[:, kc, :])

                # bulk pre-slice GEMMs into next AG input
                bulk_gemm(0, KIN, xt_loader(p), wih1_sb, nagp)
                if p >= 1:
                    bulk_gemm(1, HC, agx_tile_loader(ago_x, 0), wih2_sb, nagp)
                    bulk_gemm(2, HC, agx_tile_loader(ago_x, 1), wih3_sb, nagp)

                # projection of my xout window (real only on core 2)
                w3 = p - SKEW
                in_range = 0 <= w3 < nwin
                tgt = out if in_range else scratch_out
                r0 = w3 * rows_w if in_range else 0
                for m in range(mt):
                    ps = bpsump.tile([128, SLICE], F32, tag="bps")
                    for kc in range(HC):
                        nc.tensor.matmul(
                            ps[:, 0:OUT],
                            xout_win[:, kc, 128 * m:128 * (m + 1)],
                            wp_sb[:, kc, :],
                            start=(kc == 0), stop=(kc == HC - 1))
                    ob = bulkop.tile([128, OUT], F32, tag="projo")
                    nc.vector.tensor_copy(ob[:], ps[:, 0:OUT])
                    nc.sync.dma_start(tgt[r0 + 128 * m:r0 + 128 * (m + 1), :],
                                      ob[:])

    nc.compile()
    return nc


# ---------------- host-side glue ----------------
def prep_inputs(x, Wih1, Whh1, b1, Wih2, Whh2, b2, Wih3, Whh3, b3, Wp,
                ws=32, t=T_FULL):
    nwin = t // ws
    nprog = nwin + SKEW
    perm = gate_perm()
    bf = ml_dtypes.bfloat16

    x = np.asarray(x, np.float32)
    xT = np.ascontiguousarray(
        np.transpose(x, (2, 1, 0)).reshape(IN, t * B)).astype(bf)

    whhs = {0: np.ascontiguousarray(np.asarray(Whh1).T[:, perm]).astype(np.float32),
            1: np.ascontiguousarray(np.asarray(Whh2).T[:, perm]).astype(np.float32),
            2: np.ascontiguousarray(np.asarray(Whh3).T[:, perm]).astype(np.float32)}
    zero_whh = np.zeros((H, G), np.float32)

    wih1p = np.ascontiguousarray(np.asarray(Wih1).T[:, perm]).astype(bf)
    wih2p = np.ascontiguousarray(np.asarray(Wih2).T[:, perm]).astype(bf)
    wih3p = np.ascontiguousarray(np.asarray(Wih3).T[:, perm]).astype(bf)
    b1p = np.asarray(b1)[perm]
    b2p = np.asarray(b2)[perm]
    b3p = np.asarray(b3)[perm]

    in_maps = []
    for c in range(NCORES):
        sl = slice(SLICE * c, SLICE * (c + 1))
        al = np.full((128, 1), 1.0 if c in (1, 2) else 0.0, np.float32)
        hs = np.ones((128, nprog), np.float32)
        if c <= 2:
            hs[:, 1 + 2 * c] = 0.0
        else:
            hs[:] = 0.0
        in_maps.append({
            "xT": xT,
            "whhT": whhs.get(c, zero_whh),
            "wih1": np.ascontiguousarray(wih1p[:, sl]),
            "wih2": np.ascontiguousarray(wih2p[:, sl]),
            "wih3": np.ascontiguousarray(wih3p[:, sl]),
            "biases": np.stack([b1p[sl], b2p[sl], b3p[sl]])[None].astype(bf),
            "wpT": np.ascontiguousarray(np.asarray(Wp).T).astype(bf),
            "ident": np.eye(B, dtype=bf),
            "identf": np.eye(B, dtype=np.float32),
            "alpha": al,
            "hscale": hs,
        })
    return in_maps


_NC_CACHE = {}


def kernel(**inputs):
    if "nc" not in _NC_CACHE:
        _NC_CACHE["nc"] = build_kernel()
    nc = _NC_CACHE["nc"]
    in_maps = prep_inputs(**inputs)
    res = bass_utils.run_bass_kernel_spmd(nc, in_maps,
                                          core_ids=list(range(NCORES)))
    o = res.results[2]["out"]
    return np.ascontiguousarray(
        o.reshape(T_FULL, B, OUT).transpose(1, 0, 2)).astype(np.float32)
